# revision 27
# baseline (speedup 1.0000x reference)
"""Trainium2 Bass kernel for BondLengthConstraintEnergy.

Contract: kernel(**inputs) takes FULL unsharded inputs (as produced by the
problem's setup_inputs) and returns the FULL output [B, NCH, NRES, n_alt].

Strategy
--------
The input layout produced by setup_inputs is canonical: atom i corresponds to
(b, ch, r, a) = unravel(i) over (32, 8, 8192, 3), so the (b,ch,r,atom)->row
lookup table is exactly arange, every peptide bond (b,ch,r)->(b,ch,r+1) is
present, and the per-residue-type mean/std tables have identical rows.  Under
those conditions (verified on the host each call) the whole computation
collapses to a pure streaming stencil over coords:

  per bond r (residue r, r+1 in the same chain):
    b = C_r - CA_r          (v_cac_c)
    w = N_{r+1} - C_r       (v_cn)
    a = CA_{r+1} - N_{r+1}  (v_nca_n)
    ang1 = angle(w, a), ang2 = angle(b, -w), len = |w|
    lp_i  = min(d_i^2 / (2 var_i), -ln(EPS) - ln(sqrt(2 pi var_i)))
    out[b,ch,r,0] = (lp0+lp1+lp2) * (1 - tanh(-weight))

Angles via theta = pi/2 - atan(dot / sqrt(q - dot^2)) (exact on (0, pi)).

Device pipeline (per tile of W bonds x 128 partitions), all constants are
folded into activation scale/bias immediates:

  phase 1 (act table set: reciprocal_sqrt_and_small = square+rsqrt)
    d    = x[6:] - x[3:]                     (DVE/GpSimd split, dense)
    s    = d^2                               (Scalar Square)
    r2   = [nc2|na2|nb2]                     (3x DVE tensor_reduce X)
    p6   = d[0:6]*d[3:9]                     (DVE TT, 6-run strided)
    dots = [dot2|dot1]                       (2x DVE tensor_reduce X)
    q    = [nc2*na2|na2*nb2]                 (GpSimd TT, one [P,2W] op)
    dsq  = dots^2                            (Scalar Square)
    v    = q*(1+5e-7) - dsq                  (GpSimd STT; >=0 guaranteed)
    rs   = rsqrt(v + 1e-26), rsqrt(na2)      (Scalar Rsqrt [P,3W])
    cot  = dots * rs                         (DVE TT)
    cc   = clamp(cot, +-1.5707)              (GpSimd tensor_scalar)
    na   = na2 * rs_a                        (DVE TT)
    d0sq = (sqrt(K0)*na - sqrt(K0)*m0)^2     (Scalar Square, scale+bias imm)
    lp0  = min(d0sq, C0)                     (GpSimd tensor_scalar)
  phase 2 (act table set: sigmoid_and_others = arctan+square)
    h    = atan(cc)                          (Scalar Arctan [P,2W])
    sqi  = (sqrt(Ki)*h_i + sqrt(Ki)*bi)^2    (Scalar Square x2)
    lpi  = min(sqi, Ci)                      (GpSimd tensor_scalar x2)
    val  = (lp1+lp2)+lp0                     (DVE TT x2), DMA out

Sharding: data-parallel over batch, 4 structures per core, no communication.

If the host-side structure checks fail (inputs are not canonical), we fall
back to a faithful numpy implementation of the reference.
"""

import os
import sys

import numpy as np

for _p in ("/opt/trn_rl_repo",):
    if os.path.isdir(_p) and _p not in sys.path:
        sys.path.insert(0, _p)

# ---------------------------------------------------------------- constants
B, NCH, NRES, APR = 32, 8, 8192, 3
N_ATOMS = B * NCH * NRES * APR
NCORES = 8
B_PER_CORE = B // NCORES
RES_PER_CORE = B_PER_CORE * NCH * NRES          # 262144
ATOMS_PER_CORE = RES_PER_CORE * APR
P = 128                                          # SBUF partitions
RES_PER_PART = RES_PER_CORE // P                 # 2048
EPS = 1e-8
NEG_LOG_EPS = 18.420680743952367                 # -ln(1e-8)
R2D = 180.0 / np.pi
CLAMP = 1.5707                                   # scalar-engine atan domain
VEPS = 1e-26                                     # sqrt/reciprocal domain guard
SSAFE = 1.0 + 1e-6                               # grows q so q - dot^2 > 0

# benign pad residue (N=(0,0,0), CA=(1,0,0), C=(2,0,0)) keeps the one
# out-of-range halo bond finite; its output is overwritten on the host.
_PAD_RESIDUE = np.array([0, 0, 0, 1, 0, 0, 2, 0, 0], dtype=np.float32)

_PROGRAM = None
_PROGRAM_KEY = None

# global device-program config; _run_fast reads it too (dma_d_tiles decides
# whether the negated-coords input tensor exists / must be uploaded)
CFG = {"dma_d_tiles": (0, 1, 2, 3)}

# default engine for routable ops: 'd' = Vector/DVE, 'g' = GpSimd
_ENG_DEFAULT = {
    "q": "d",
    "dsq": "g",
    "v": "g",
    "cot": "d",
    "cc": "g",
    "lp": "g",
    "s12": "d",
    "val": "d",
}


def _register_custom_ops():
    """Register fused custom-DVE ops (in-process; table is written per-NEFF).

    V_FUSE_ANT:    out = in0 - in1^2           (q - dot^2 in one pass)
    MUL_CLAMP_ANT: out = clip(in0*in1, +-s0)   (cot + atan-domain clamp)
    MIN_ADD_ANT:   out = min(in0, s0) + in1    (lp clip + energy sum chain)
    """
    from concourse import dve_ops
    from concourse.dve_spec import (
        Spec, Src0, Src1, C0, sq, minn, maxx, lower, _has_src1)
    from concourse.dve_uop import DveOpSpec

    def ref_vfuse(in0, in1, c0, c1, c2):
        return (in0 - np.square(in1.astype(np.float32))).astype(np.float32)

    def ref_mulclamp(in0, in1, c0, c1, c2):
        c = np.float32(c0) if not isinstance(c0, np.ndarray) else c0
        return np.clip((in0.astype(np.float32) * in1).astype(np.float32),
                       -c, c).astype(np.float32)

    def ref_minadd(in0, in1, c0, c1, c2):
        return (np.minimum(in0.astype(np.float32), c0) + in1).astype(
            np.float32)

    specs = {
        "V_FUSE_ANT": Spec(body=Src0 - sq(Src1), reference=ref_vfuse),
        "MUL_CLAMP_ANT": Spec(body=minn(maxx(Src0 * Src1, -C0), C0),
                              reference=ref_mulclamp),
        "MIN_ADD_ANT": Spec(body=minn(Src0, C0) + Src1, reference=ref_minadd),
    }
    made = {}
    for name, spec in specs.items():
        existing = next((o for o in dve_ops.OPS if o.name == name), None)
        if existing is not None:
            made[name] = existing
            continue
        row = dve_ops._CUSTOM_DVE_ROW_BASE + len(dve_ops.OPS)
        assert row < 0x20
        dve_ops._SUB_OPCODE_FOR_NAME[name] = row
        shas = {}
        for ver in ("v3", "v4"):
            try:
                sl = DveOpSpec(name=name, opcode=row,
                               uops=lower(spec, ver=ver),
                               rd1_en=_has_src1(spec))
                shas[ver] = sl.sha(ver)
            except Exception:
                pass
        op = dve_ops.DveOp(name, spec, subdim=False, uops_sha=shas)
        dve_ops.OPS.append(op)
        dve_ops.CUSTOM_DVE_SPECS[name] = spec
        made[name] = op
    return made


def _consts(mean_row, std_row, weight0):
    """Scalar immediates for the device program (f folded into K*, C*)."""
    m = np.asarray(mean_row, dtype=np.float64)
    s = np.asarray(std_row, dtype=np.float64)
    f = 1.0 - np.tanh(-float(weight0))
    var = s * s
    clip = (NEG_LOG_EPS - 0.5 * np.log(2.0 * np.pi * var)) * f
    K = np.array([1.0 / (2.0 * var[0]),
                  (R2D * R2D) / (2.0 * var[1]),
                  (R2D * R2D) / (2.0 * var[2])]) * f
    return {
        "sk0": float(np.sqrt(K[0])),
        "b0": float(-np.sqrt(K[0]) * m[0]),
        "c0": float(clip[0]),
        "sk1": float(np.sqrt(K[1])),
        "b1": float(np.sqrt(K[1]) * (m[1] - 90.0) / R2D),
        "c1": float(clip[1]),
        "sk2": float(np.sqrt(K[2])),
        "b2": float(np.sqrt(K[2]) * (90.0 - m[2]) / R2D),
        "c2": float(clip[2]),
    }


# ---------------------------------------------------------------- device IR
def _build_program(con, reps=1, cfg=None):
    """Build + compile the per-core Bass/Tile program (identical on all cores).

    `con` is the dict of scalar immediates from _consts.
    """
    import concourse.bacc as bacc
    import concourse.bass as bass
    import concourse.mybir as mybir
    import concourse.tile as tile

    import bass_rust

    cfg = dict(cfg or {})
    W = cfg.get("W", 512)
    NT = RES_PER_PART // W
    assert RES_PER_PART % W == 0
    # fraction of the 9W d-columns computed on DVE (rest on GpSimd)
    d_dve = cfg.get("d_dve", 0.0)
    kd = int(round(9 * W * d_dve))
    kd -= kd % 3
    fence = cfg.get("fence", True)
    fuse = _register_custom_ops() if cfg.get("fuse", True) else None

    dt = mybir.dt
    Alu = mybir.AluOpType
    Act = mybir.ActivationFunctionType

    nc = bacc.Bacc(
        "TRN2",
        target_bir_lowering=False,
        debug=False,
        enable_asserts=False,
        num_devices=NCORES,
    )

    dma_d_tiles = set(cfg.get("dma_d_tiles", ()))

    xin = nc.dram_tensor("xin", [(RES_PER_CORE + 1) * 9], dt.float32,
                         kind="ExternalInput")
    xng = None
    if dma_d_tiles:
        # host-negated copy of xin: lets the SDMA CCE compute
        # d = x[6:] + (-x)[3:] during the load (accum only supports add)
        xng = nc.dram_tensor("xng", [(RES_PER_CORE + 1) * 9], dt.float32,
                             kind="ExternalInput")
    out = nc.dram_tensor("out", [RES_PER_CORE], dt.float32,
                         kind="ExternalOutput")

    # activation float biases lower to per-partition const APs; register ours
    for _v in (VEPS, con["b0"], con["b1"], con["b2"]):
        if (dt.float32, _v) not in nc.const_aps.aps:
            _t = nc.alloc_sbuf_tensor(f"const-f32-{_v}", [128, 1], dt.float32)
            nc.gpsimd.memset(_t.ap(), _v)
            nc.const_aps.aps[(dt.float32, _v)] = _t.ap()
    nc.all_engine_barrier()

    FW = 9 * W
    XW = 9 * (W + 1)

    # engine router: 'd' -> DVE, 'g' -> GpSimd, with optional column split
    def eng(name):
        return nc.vector if cfg.get(name, _ENG_DEFAULT[name]) == "d" else nc.gpsimd

    with tile.TileContext(nc) as tc:
        with (
            tc.tile_pool(name="xpool", bufs=cfg.get("xbufs", 2)) as xpool,
            tc.tile_pool(name="dpool", bufs=cfg.get("dbufs", 2)) as dpool,
            tc.tile_pool(name="ppool", bufs=cfg.get("pbufs", 2)) as ppool,
            tc.tile_pool(name="mid2", bufs=cfg.get("midbufs", 2)) as mid2,
            tc.tile_pool(name="mid1", bufs=cfg.get("mid1bufs", 1)) as mid1,
            tc.tile_pool(name="xph", bufs=NT) as xph,     # crosses phase bound
            tc.tile_pool(name="ph2", bufs=cfg.get("ph2bufs", 1)) as ph2,
            tc.tile_pool(name="vpool", bufs=cfg.get("vbufs", 2)) as vpool,
        ):
            ph1_act = []

            def emit_ph1(t):
                base = P * W * t
                xt = xpool.tile([P, XW], dt.float32, tag="x")
                dt_ = dpool.tile([P, FW], dt.float32, tag="d")
                if t in dma_d_tiles:
                    # d computed by the DMA engines: load x[6:], accum -x[3:].
                    # CCE accumulate descriptors cap at 2048 elems -> chunk.
                    d = xt[:, 0:FW]
                    nc.gpsimd.dma_start(
                        d, bass.AP(xin, base * 9 + 6, [[FW, P], [1, FW]]))
                    CH = FW // 3
                    for k in range(3):
                        nc.gpsimd.dma_start(
                            xt[:, CH * k:CH * (k + 1)],
                            bass.AP(xng, base * 9 + 3 + CH * k,
                                    [[FW, P], [1, CH]]),
                            accum_op=Alu.add)
                    s = dt_[:, :]
                else:
                    x = xt
                    nc.sync.dma_start(
                        x[:, :], bass.AP(xin, base * 9, [[FW, P], [1, XW]]))
                    # d[i] = x[i+6] - x[i+3]; per bond j:
                    #   d[9j+0..2]=b, d[9j+3..5]=w, d[9j+6..8]=a
                    d = dt_[:, :]
                    if kd > 0:
                        nc.vector.tensor_sub(d[:, 0:kd], x[:, 6:6 + kd],
                                             x[:, 3:3 + kd])
                    if kd < FW:
                        nc.gpsimd.tensor_sub(d[:, kd:FW], x[:, 6 + kd:6 + FW],
                                             x[:, 3 + kd:3 + FW])
                    # s overwrites the (dead) x tile's space
                    s = x[:, 0:FW]

                # s grouped by vector: s[ti*3W + 3w + k] = (SSAFE*d[9w+3ti+k])^2
                # (ti=0 -> b, 1 -> w, 2 -> a); SSAFE keeps v = q - dot^2 > 0
                d3 = d.rearrange("p (w k) -> p w k", k=9)
                s3 = s.rearrange("p (t w k) -> p t w k", t=3, k=3)
                for ti in range(3):
                    ph1_act.append(
                        nc.scalar.activation(s3[:, ti, :, :],
                                             d3[:, :, 3 * ti:3 * ti + 3],
                                             Act.Square, scale=SSAFE))

                # r2 planar: [nc2 | na2 | nb2]  (dense windowed reduces)
                r2 = mid2.tile([P, 3 * W], dt.float32, tag="r2")
                for ti in range(3):
                    nc.vector.tensor_reduce(
                        r2[:, ti * W:(ti + 1) * W].rearrange(
                            "p (w o) -> p w o", o=1),
                        s3[:, ti, :, :], axis=mybir.AxisListType.X, op=Alu.add)
                na2 = r2[:, W:2 * W]

                # p6[6j+m] = d[9j+m]*d[9j+m+3]  (m=0..2 -> dot2, 3..5 -> dot1)
                p6 = ppool.tile([P, 6 * W], dt.float32, tag="p6")
                p6v = p6[:, :].rearrange("p (w k) -> p w k", k=6)
                gfrac = cfg.get("p6_gps", 0.5)
                kp = int(round(W * gfrac))
                if kp > 0:
                    nc.gpsimd.tensor_tensor(p6v[:, 0:kp], d3[:, 0:kp, 0:6],
                                            d3[:, 0:kp, 3:9], op=Alu.mult)
                if kp < W:
                    nc.vector.tensor_tensor(p6v[:, kp:W], d3[:, kp:W, 0:6],
                                            d3[:, kp:W, 3:9], op=Alu.mult)

                # dots planar: [dot2 | dot1]
                pvv = p6[:, :].rearrange("p (w t k) -> p w t k", t=2, k=3)
                dots = mid2.tile([P, 2 * W], dt.float32, tag="dots")
                for ti in range(2):
                    nc.vector.tensor_reduce(
                        dots[:, ti * W:(ti + 1) * W].rearrange(
                            "p (w o) -> p w o", o=1),
                        pvv[:, :, ti, :], axis=mybir.AxisListType.X,
                        op=Alu.add)

                # q planar: [nc2*na2 | na2*nb2]  (dense overlapping slices)
                q = mid2.tile([P, 2 * W], dt.float32, tag="q")
                eng("q").tensor_tensor(q[:, :], r2[:, 0:2 * W],
                                       r2[:, W:3 * W], op=Alu.mult)

                # v = q - dots^2  (> 0 thanks to SSAFE)
                v = mid1.tile([P, 2 * W], dt.float32, tag="v")
                if fuse:
                    nc.vector._custom_dve(fuse["V_FUSE_ANT"], out=v[:, :],
                                          in0=q[:, :], in1=dots[:, :])
                else:
                    dsq = mid1.tile([P, 2 * W], dt.float32, tag="dsq")
                    eng("dsq").tensor_tensor(dsq[:, :], dots[:, :],
                                             dots[:, :], op=Alu.mult)
                    eng("v").tensor_tensor(v[:, :], q[:, :], dsq[:, :],
                                           op=Alu.subtract)

                # sqv = sqrt(v); rcp = 1/sqv (fast custom-DVE reciprocal)
                sqv = mid1.tile([P, 2 * W], dt.float32, tag="sqv")
                ph1_act.append(
                    nc.scalar.activation(sqv[:, :], v[:, :], Act.Sqrt,
                                         bias=VEPS))
                rcp = mid1.tile([P, 2 * W], dt.float32, tag="rcp")
                nc.vector.reciprocal_approx_fast(rcp[:, :], sqv[:, :])

                # cot = dots*rcp, clamped into the carry: [cc1 | cc2 | lp0]
                carry = xph.tile([P, 3 * W], dt.float32, tag="carry")
                if fuse:
                    nc.vector._custom_dve(
                        fuse["MUL_CLAMP_ANT"], out=carry[:, 0:W],
                        in0=dots[:, W:2 * W], in1=rcp[:, W:2 * W], s0=CLAMP)
                    nc.vector._custom_dve(
                        fuse["MUL_CLAMP_ANT"], out=carry[:, W:2 * W],
                        in0=dots[:, 0:W], in1=rcp[:, 0:W], s0=CLAMP)
                else:
                    cot = mid1.tile([P, 2 * W], dt.float32, tag="cot")
                    eng("cot").tensor_tensor(cot[:, :], dots[:, :], rcp[:, :],
                                             op=Alu.mult)
                    eng("cc").tensor_scalar(carry[:, 0:W], cot[:, W:2 * W],
                                            CLAMP, -CLAMP, op0=Alu.min,
                                            op1=Alu.max)
                    eng("cc").tensor_scalar(carry[:, W:2 * W], cot[:, 0:W],
                                            CLAMP, -CLAMP, op0=Alu.min,
                                            op1=Alu.max)

                # length term: na = sqrt(na2); d0sq = (sk0*na + b0)^2;
                # lp0 = min(d0sq, c0)  (max-with-0 is a free no-op 2nd op)
                na = mid1.tile([P, W], dt.float32, tag="na")
                ph1_act.append(
                    nc.scalar.activation(na[:, :], na2, Act.Sqrt))
                d0sq = mid1.tile([P, W], dt.float32, tag="d0sq")
                ph1_act.append(
                    nc.scalar.activation(d0sq[:, :], na[:, :], Act.Square,
                                         bias=con["b0"], scale=con["sk0"]))
                eng("lp").tensor_scalar(carry[:, 2 * W:3 * W], d0sq[:, :],
                                        con["c0"], 0.0, op0=Alu.min,
                                        op1=Alu.max)
                return carry

            def emit_ph2(t, carry):
                h = ph2.tile([P, 2 * W], dt.float32, tag="h")
                h_inst = nc.scalar.activation(h[:, :], carry[:, 0:2 * W],
                                              Act.Arctan)
                if fence:
                    for a in ph1_act:
                        bass_rust.add_dep_helper(
                            h_inst.ins, a.ins,
                            reason="act set fence: sqrt-set before arctan")
                # sq_i = (sk_i*h_i + b_i)^2  (h1 in [0:W], h2 in [W:2W])
                sq = ph2.tile([P, 2 * W], dt.float32, tag="sq")
                nc.scalar.activation(sq[:, 0:W], h[:, 0:W], Act.Square,
                                     bias=con["b1"], scale=con["sk1"])
                nc.scalar.activation(sq[:, W:2 * W], h[:, W:2 * W], Act.Square,
                                     bias=con["b2"], scale=con["sk2"])
                val = vpool.tile([P, W], dt.float32, tag="val")
                if fuse:
                    # val = min(sq2,c2) + (min(sq1,c1) + lp0)
                    t1 = ph2.tile([P, W], dt.float32, tag="t1")
                    nc.vector._custom_dve(
                        fuse["MIN_ADD_ANT"], out=t1[:, :], in0=sq[:, 0:W],
                        in1=carry[:, 2 * W:3 * W], s0=con["c1"])
                    nc.vector._custom_dve(
                        fuse["MIN_ADD_ANT"], out=val[:, :],
                        in0=sq[:, W:2 * W], in1=t1[:, :], s0=con["c2"])
                else:
                    lp = ph2.tile([P, 2 * W], dt.float32, tag="lp")
                    eng("lp").tensor_scalar(lp[:, 0:W], sq[:, 0:W], con["c1"],
                                            0.0, op0=Alu.min, op1=Alu.max)
                    eng("lp").tensor_scalar(lp[:, W:2 * W], sq[:, W:2 * W],
                                            con["c2"], 0.0, op0=Alu.min,
                                            op1=Alu.max)
                    s12 = ph2.tile([P, W], dt.float32, tag="s12")
                    eng("s12").tensor_tensor(s12[:, :], lp[:, 0:W],
                                             lp[:, W:2 * W], op=Alu.add)
                    eng("val").tensor_tensor(val[:, :], s12[:, :],
                                             carry[:, 2 * W:3 * W],
                                             op=Alu.add)
                dst = bass.AP(out, P * W * t, [[W, P], [1, W]])
                nc.sync.dma_start(dst, val[:, :])

            def _body():
                carries = []
                for t in range(NT):
                    carries.append(emit_ph1(t))
                for t in range(NT):
                    emit_ph2(t, carries[t])

            if reps == 1:
                _body()
            else:
                with tc.For_i(0, reps, 1):
                    _body()

    nc.compile()
    return nc


def _get_program(con, cfg=None):
    global _PROGRAM, _PROGRAM_KEY
    if cfg is None:
        cfg = CFG
    key = (tuple(sorted(con.items())), repr(sorted(cfg.items())))
    if _PROGRAM is None or _PROGRAM_KEY != key:
        _PROGRAM = _build_program(con, cfg=cfg)
        _PROGRAM_KEY = key
    return _PROGRAM


# ---------------------------------------------------------------- host side
def _is_canonical(ad, coords, mean, std):
    if ad.shape != (N_ATOMS, 5) or coords.shape != (N_ATOMS, 3):
        return False
    if mean.shape != (20, 3) or std.shape != (20, 3):
        return False
    if not (np.all(mean == mean[0:1]) and np.all(std == std[0:1])):
        return False
    if not np.all(std[0] > 0):
        return False
    a5 = ad.reshape(B, NCH, NRES, APR, 5)
    if not np.all(a5[..., 0] == np.arange(B, dtype=ad.dtype)[:, None, None, None]):
        return False
    if not np.all(a5[..., 1] == np.arange(NCH, dtype=ad.dtype)[:, None, None]):
        return False
    if not np.all(a5[..., 2] == np.arange(NRES, dtype=ad.dtype)[:, None]):
        return False
    if not np.all(a5[..., 4] == np.arange(APR, dtype=ad.dtype)):
        return False
    if not np.isfinite(coords).all() or np.abs(coords).max() >= 1e4:
        return False
    # all bond-geometry norms must clear the reference's EPS mask, so the
    # device kernel can skip mask arithmetic entirely
    r = coords.reshape(B, NCH, NRES, 9)
    w = r[:, :, 1:, 0:3] - r[:, :, :-1, 6:9]
    a = r[:, :, 1:, 3:6] - r[:, :, 1:, 0:3]
    bb = r[:, :, :-1, 6:9] - r[:, :, :-1, 3:6]
    mn = min(
        (w * w).sum(-1).min(),
        (a * a).sum(-1).min(),
        (bb * bb).sum(-1).min(),
    )
    return bool(mn > 1.1e-16)


def _run_fast(coords, mean, std, weight, n_alt):
    from concourse import bass_utils

    con = _consts(mean[0], std[0], weight[0])
    nc = _get_program(con)
    need_xng = bool(CFG.get("dma_d_tiles"))
    cflat = np.ascontiguousarray(coords.reshape(-1), dtype=np.float32)
    in_maps = []
    for c in range(NCORES):
        shard = np.empty(((RES_PER_CORE + 1) * 9,), dtype=np.float32)
        shard[:-9] = cflat[c * ATOMS_PER_CORE * 3:(c + 1) * ATOMS_PER_CORE * 3]
        shard[-9:] = _PAD_RESIDUE
        m = {"xin": shard}
        if need_xng:
            m["xng"] = -shard
        in_maps.append(m)

    res = bass_utils.run_bass_kernel_spmd(nc, in_maps,
                                          core_ids=list(range(NCORES)))
    parts = [np.asarray(res.results[c]["out"], dtype=np.float32)
             for c in range(NCORES)]
    e = np.concatenate(parts).reshape(B, NCH, NRES)
    e[:, :, NRES - 1] = 0.0          # no bond out of the last residue
    full = np.zeros((B, NCH, NRES, n_alt), dtype=np.float32)
    full[..., 0] = e
    return full


# ------------------------------------------------------------ numpy fallback
def _fallback(ad, coords, alternatives, weight, mean, std):
    """Faithful numpy port of the jax reference (incl. OOB drop/clamp)."""
    n_alt = alternatives.shape[-1]
    batch, chain, resnum = ad[:, 0], ad[:, 1], ad[:, 2]
    resname, at_name = ad[:, 3], ad[:, 4]
    n = ad.shape[0]

    table = np.full((B, NCH, NRES, APR), -1, dtype=np.int32)
    ok = ((batch >= 0) & (batch < B) & (chain >= 0) & (chain < NCH)
          & (resnum >= 0) & (resnum < NRES) & (at_name >= 0) & (at_name < APR))
    idx = np.arange(n, dtype=np.int32)
    table[batch[ok], chain[ok], resnum[ok], at_name[ok]] = idx[ok]

    c_idx = table[:, :, :-1, 2].reshape(-1)
    n_idx = table[:, :, 1:, 0].reshape(-1)
    cac_idx = table[:, :, :-1, 1].reshape(-1)
    can_idx = table[:, :, 1:, 1].reshape(-1)
    valid_idx = (c_idx >= 0) & (n_idx >= 0) & (cac_idx >= 0) & (can_idx >= 0)
    safe = lambda i: np.where(i >= 0, i, 0)

    co = coords.astype(np.float32)
    c_xyz = co[safe(c_idx)]
    n_xyz = co[safe(n_idx)]
    cac_xyz = co[safe(cac_idx)]
    can_xyz = co[safe(can_idx)]

    v_cn = n_xyz - c_xyz
    v_nca = can_xyz - n_xyz
    v_cac = c_xyz - cac_xyz

    def ang_deg(a, b):
        na = np.sqrt((a * a).sum(-1))
        nb = np.sqrt((b * b).sum(-1))
        mask = (na > EPS) & (nb > EPS)
        cos = np.clip((a * b).sum(-1) / (na * nb + EPS), -1.0, 1.0)
        return np.degrees(np.arccos(cos)).astype(np.float32), mask

    ang1, m1 = ang_deg(v_cn, v_nca)
    ang2, m2 = ang_deg(v_cac, -v_cn)
    bond_len = np.sqrt((v_cn * v_cn).sum(-1))
    valid = valid_idx & m1 & m2

    geom = np.stack([bond_len, ang1, ang2], axis=-1)
    seq = np.clip(resname[safe(c_idx)], 0, 19)
    var = (std.astype(np.float32)[seq]) ** 2
    denom = np.sqrt(2.0 * np.pi * var).astype(np.float32)
    num = np.exp(-((geom - mean.astype(np.float32)[seq]) ** 2) / (2.0 * var))
    log_prob = -(np.log(np.clip(num / denom, EPS, None)) + np.log(denom))
    scores = log_prob.sum(-1)

    f = np.float32(1.0 - np.tanh(-np.float32(weight[0])))
    val = np.where(valid, scores * f, 0.0).astype(np.float32)

    b_c = batch[safe(c_idx)]
    ch_c = chain[safe(c_idx)]
    r_c = resnum[safe(c_idx)]
    resi = np.zeros((B, NCH, NRES, n_alt), dtype=np.float32)
    ok2 = ((b_c >= 0) & (b_c < B) & (ch_c >= 0) & (ch_c < NCH)
           & (r_c >= 0) & (r_c < NRES))
    resi[b_c[ok2], ch_c[ok2], r_c[ok2], 0] = val[ok2]
    return resi


# ----------------------------------------------------------------- entry
def kernel(atom_description, coords, alternatives, weight, mean, std):
    ad = np.asarray(atom_description)
    co = np.asarray(coords, dtype=np.float32)
    al = np.asarray(alternatives)
    wt = np.asarray(weight, dtype=np.float32)
    mn = np.asarray(mean, dtype=np.float32)
    sd = np.asarray(std, dtype=np.float32)

    if _is_canonical(ad, co, mn, sd):
        return _run_fast(co, mn, sd, wt, al.shape[-1])
    return _fallback(ad, co, al, wt, mn, sd)


# revision 28
# speedup vs baseline: 1.1612x; 1.1612x over previous
"""Trainium2 Bass kernel for BondLengthConstraintEnergy.

Contract: kernel(**inputs) takes FULL unsharded inputs (as produced by the
problem's setup_inputs) and returns the FULL output [B, NCH, NRES, n_alt].

Strategy
--------
The input layout produced by setup_inputs is canonical: atom i corresponds to
(b, ch, r, a) = unravel(i) over (32, 8, 8192, 3), so the (b,ch,r,atom)->row
lookup table is exactly arange, every peptide bond (b,ch,r)->(b,ch,r+1) is
present, and the per-residue-type mean/std tables have identical rows.  Under
those conditions (verified on the host each call) the whole computation
collapses to a pure streaming stencil over coords:

  per bond r (residue r, r+1 in the same chain):
    b = C_r - CA_r          (v_cac_c)
    w = N_{r+1} - C_r       (v_cn)
    a = CA_{r+1} - N_{r+1}  (v_nca_n)
    ang1 = angle(w, a), ang2 = angle(b, -w), len = |w|
    lp_i  = min(d_i^2 / (2 var_i), -ln(EPS) - ln(sqrt(2 pi var_i)))
    out[b,ch,r,0] = (lp0+lp1+lp2) * (1 - tanh(-weight))

Angles via theta = pi/2 - atan(dot / sqrt(q - dot^2)) (exact on (0, pi)).

Device pipeline (per tile of W bonds x 128 partitions), all constants are
folded into activation scale/bias immediates:

  phase 1 (act table set: reciprocal_sqrt_and_small = square+rsqrt)
    d    = x[6:] - x[3:]                     (DVE/GpSimd split, dense)
    s    = d^2                               (Scalar Square)
    r2   = [nc2|na2|nb2]                     (3x DVE tensor_reduce X)
    p6   = d[0:6]*d[3:9]                     (DVE TT, 6-run strided)
    dots = [dot2|dot1]                       (2x DVE tensor_reduce X)
    q    = [nc2*na2|na2*nb2]                 (GpSimd TT, one [P,2W] op)
    dsq  = dots^2                            (Scalar Square)
    v    = q*(1+5e-7) - dsq                  (GpSimd STT; >=0 guaranteed)
    rs   = rsqrt(v + 1e-26), rsqrt(na2)      (Scalar Rsqrt [P,3W])
    cot  = dots * rs                         (DVE TT)
    cc   = clamp(cot, +-1.5707)              (GpSimd tensor_scalar)
    na   = na2 * rs_a                        (DVE TT)
    d0sq = (sqrt(K0)*na - sqrt(K0)*m0)^2     (Scalar Square, scale+bias imm)
    lp0  = min(d0sq, C0)                     (GpSimd tensor_scalar)
  phase 2 (act table set: sigmoid_and_others = arctan+square)
    h    = atan(cc)                          (Scalar Arctan [P,2W])
    sqi  = (sqrt(Ki)*h_i + sqrt(Ki)*bi)^2    (Scalar Square x2)
    lpi  = min(sqi, Ci)                      (GpSimd tensor_scalar x2)
    val  = (lp1+lp2)+lp0                     (DVE TT x2), DMA out

Sharding: data-parallel over batch, 4 structures per core, no communication.

If the host-side structure checks fail (inputs are not canonical), we fall
back to a faithful numpy implementation of the reference.
"""

import os
import sys

import numpy as np

for _p in ("/opt/trn_rl_repo",):
    if os.path.isdir(_p) and _p not in sys.path:
        sys.path.insert(0, _p)

# ---------------------------------------------------------------- constants
B, NCH, NRES, APR = 32, 8, 8192, 3
N_ATOMS = B * NCH * NRES * APR
NCORES = 8
B_PER_CORE = B // NCORES
RES_PER_CORE = B_PER_CORE * NCH * NRES          # 262144
ATOMS_PER_CORE = RES_PER_CORE * APR
P = 128                                          # SBUF partitions
RES_PER_PART = RES_PER_CORE // P                 # 2048
EPS = 1e-8
NEG_LOG_EPS = 18.420680743952367                 # -ln(1e-8)
R2D = 180.0 / np.pi
CLAMP = 1.5707                                   # scalar-engine atan domain
VEPS = 1e-26                                     # sqrt/reciprocal domain guard
SSAFE = 1.0 + 1e-6                               # grows q so q - dot^2 > 0

# benign pad residue (N=(0,0,0), CA=(1,0,0), C=(2,0,0)) keeps the one
# out-of-range halo bond finite; its output is overwritten on the host.
_PAD_RESIDUE = np.array([0, 0, 0, 1, 0, 0, 2, 0, 0], dtype=np.float32)

_PROGRAM = None
_PROGRAM_KEY = None

# global device-program config; _run_fast reads it too (dma_d_tiles decides
# whether the negated-coords input tensor exists / must be uploaded)
CFG = {"dma_d_tiles": (0, 1, 2, 3)}

# default engine for routable ops: 'd' = Vector/DVE, 'g' = GpSimd
_ENG_DEFAULT = {
    "q": "d",
    "dsq": "g",
    "v": "g",
    "cot": "d",
    "cc": "g",
    "lp": "g",
    "s12": "d",
    "val": "d",
}


def _register_custom_ops():
    """Register fused custom-DVE ops (in-process; table is written per-NEFF).

    V_FUSE_ANT:    out = in0 - in1^2           (q - dot^2 in one pass)
    MUL_CLAMP_ANT: out = clip(in0*in1, +-s0)   (cot + atan-domain clamp)
    MIN_ADD_ANT:   out = min(in0, s0) + in1    (lp clip + energy sum chain)
    """
    from concourse import dve_ops
    from concourse.dve_spec import (
        Spec, Src0, Src1, C0, sq, minn, maxx, lower, _has_src1)
    from concourse.dve_uop import DveOpSpec

    def ref_vfuse(in0, in1, c0, c1, c2):
        return (in0 - np.square(in1.astype(np.float32))).astype(np.float32)

    def ref_mulclamp(in0, in1, c0, c1, c2):
        c = np.float32(c0) if not isinstance(c0, np.ndarray) else c0
        return np.clip((in0.astype(np.float32) * in1).astype(np.float32),
                       -c, c).astype(np.float32)

    def ref_minadd(in0, in1, c0, c1, c2):
        return (np.minimum(in0.astype(np.float32), c0) + in1).astype(
            np.float32)

    specs = {
        "V_FUSE_ANT": Spec(body=Src0 - sq(Src1), reference=ref_vfuse),
        "MUL_CLAMP_ANT": Spec(body=minn(maxx(Src0 * Src1, -C0), C0),
                              reference=ref_mulclamp),
        "MIN_ADD_ANT": Spec(body=minn(Src0, C0) + Src1, reference=ref_minadd),
    }
    made = {}
    for name, spec in specs.items():
        existing = next((o for o in dve_ops.OPS if o.name == name), None)
        if existing is not None:
            made[name] = existing
            continue
        row = dve_ops._CUSTOM_DVE_ROW_BASE + len(dve_ops.OPS)
        assert row < 0x20
        dve_ops._SUB_OPCODE_FOR_NAME[name] = row
        shas = {}
        for ver in ("v3", "v4"):
            try:
                sl = DveOpSpec(name=name, opcode=row,
                               uops=lower(spec, ver=ver),
                               rd1_en=_has_src1(spec))
                shas[ver] = sl.sha(ver)
            except Exception:
                pass
        op = dve_ops.DveOp(name, spec, subdim=False, uops_sha=shas)
        dve_ops.OPS.append(op)
        dve_ops.CUSTOM_DVE_SPECS[name] = spec
        made[name] = op
    return made


def _consts(mean_row, std_row, weight0):
    """Scalar immediates for the device program (f folded into K*, C*)."""
    m = np.asarray(mean_row, dtype=np.float64)
    s = np.asarray(std_row, dtype=np.float64)
    f = 1.0 - np.tanh(-float(weight0))
    var = s * s
    clip = (NEG_LOG_EPS - 0.5 * np.log(2.0 * np.pi * var)) * f
    K = np.array([1.0 / (2.0 * var[0]),
                  (R2D * R2D) / (2.0 * var[1]),
                  (R2D * R2D) / (2.0 * var[2])]) * f
    return {
        "sk0": float(np.sqrt(K[0])),
        "b0": float(-np.sqrt(K[0]) * m[0]),
        "c0": float(clip[0]),
        "sk1": float(np.sqrt(K[1])),
        "b1": float(np.sqrt(K[1]) * (m[1] - 90.0) / R2D),
        "c1": float(clip[1]),
        "sk2": float(np.sqrt(K[2])),
        "b2": float(np.sqrt(K[2]) * (90.0 - m[2]) / R2D),
        "c2": float(clip[2]),
    }


# ---------------------------------------------------------------- device IR
def _build_program(con, reps=1, cfg=None):
    """Build + compile the per-core Bass/Tile program (identical on all cores).

    `con` is the dict of scalar immediates from _consts.
    """
    import concourse.bacc as bacc
    import concourse.bass as bass
    import concourse.mybir as mybir
    import concourse.tile as tile

    import bass_rust

    cfg = dict(cfg or {})
    W = cfg.get("W", 512)
    NT = RES_PER_PART // W
    assert RES_PER_PART % W == 0
    # fraction of the 9W d-columns computed on DVE (rest on GpSimd)
    d_dve = cfg.get("d_dve", 0.0)
    kd = int(round(9 * W * d_dve))
    kd -= kd % 3
    fence = cfg.get("fence", True)
    fuse = _register_custom_ops() if cfg.get("fuse", True) else None

    dt = mybir.dt
    Alu = mybir.AluOpType
    Act = mybir.ActivationFunctionType

    nc = bacc.Bacc(
        "TRN2",
        target_bir_lowering=False,
        debug=False,
        enable_asserts=False,
        num_devices=NCORES,
    )

    dma_d_tiles = set(cfg.get("dma_d_tiles", ()))

    xin = nc.dram_tensor("xin", [(RES_PER_CORE + 1) * 9], dt.float32,
                         kind="ExternalInput")
    xng = None
    if dma_d_tiles:
        # host-negated copy of xin: lets the SDMA CCE compute
        # d = x[6:] + (-x)[3:] during the load (accum only supports add)
        xng = nc.dram_tensor("xng", [(RES_PER_CORE + 1) * 9], dt.float32,
                             kind="ExternalInput")
    out = nc.dram_tensor("out", [RES_PER_CORE], dt.float32,
                         kind="ExternalOutput")

    # activation float biases lower to per-partition const APs; register ours
    for _v in (VEPS, con["b0"], con["b1"], con["b2"]):
        if (dt.float32, _v) not in nc.const_aps.aps:
            _t = nc.alloc_sbuf_tensor(f"const-f32-{_v}", [128, 1], dt.float32)
            nc.gpsimd.memset(_t.ap(), _v)
            nc.const_aps.aps[(dt.float32, _v)] = _t.ap()
    nc.all_engine_barrier()

    FW = 9 * W
    XW = 9 * (W + 1)

    # engine router: 'd' -> DVE, 'g' -> GpSimd, with optional column split
    def eng(name):
        return nc.vector if cfg.get(name, _ENG_DEFAULT[name]) == "d" else nc.gpsimd

    with tile.TileContext(nc) as tc:
        with (
            tc.tile_pool(name="xpool", bufs=cfg.get("xbufs", 2)) as xpool,
            tc.tile_pool(name="dpool", bufs=cfg.get("dbufs", 2)) as dpool,
            tc.tile_pool(name="ppool", bufs=cfg.get("pbufs", 2)) as ppool,
            tc.tile_pool(name="mid2", bufs=cfg.get("midbufs", 2)) as mid2,
            tc.tile_pool(name="mid1", bufs=cfg.get("mid1bufs", 1)) as mid1,
            tc.tile_pool(name="xph", bufs=NT) as xph,     # crosses phase bound
            tc.tile_pool(name="ph2", bufs=cfg.get("ph2bufs", 1)) as ph2,
            tc.tile_pool(name="vpool", bufs=cfg.get("vbufs", 2)) as vpool,
        ):
            ph1_act = []

            def emit_ph1(t):
                base = P * W * t
                xt = xpool.tile([P, XW], dt.float32, tag="x")
                dt_ = dpool.tile([P, FW], dt.float32, tag="d")
                if t in dma_d_tiles:
                    # d computed by the DMA engines: load x[6:], accum -x[3:].
                    # CCE accumulate descriptors cap at 2048 elems -> chunk.
                    d = xt[:, 0:FW]
                    nc.gpsimd.dma_start(
                        d, bass.AP(xin, base * 9 + 6, [[FW, P], [1, FW]]))
                    CH = FW // 3
                    for k in range(3):
                        nc.gpsimd.dma_start(
                            xt[:, CH * k:CH * (k + 1)],
                            bass.AP(xng, base * 9 + 3 + CH * k,
                                    [[FW, P], [1, CH]]),
                            accum_op=Alu.add)
                    s = dt_[:, :]
                else:
                    x = xt
                    nc.sync.dma_start(
                        x[:, :], bass.AP(xin, base * 9, [[FW, P], [1, XW]]))
                    # d[i] = x[i+6] - x[i+3]; per bond j:
                    #   d[9j+0..2]=b, d[9j+3..5]=w, d[9j+6..8]=a
                    d = dt_[:, :]
                    if kd > 0:
                        nc.vector.tensor_sub(d[:, 0:kd], x[:, 6:6 + kd],
                                             x[:, 3:3 + kd])
                    if kd < FW:
                        nc.gpsimd.tensor_sub(d[:, kd:FW], x[:, 6 + kd:6 + FW],
                                             x[:, 3 + kd:3 + FW])
                    # s overwrites the (dead) x tile's space
                    s = x[:, 0:FW]

                # s grouped by vector: s[ti*3W + 3w + k] = (SSAFE*d[9w+3ti+k])^2
                # (ti=0 -> b, 1 -> w, 2 -> a); SSAFE keeps v = q - dot^2 > 0
                d3 = d.rearrange("p (w k) -> p w k", k=9)
                s3 = s.rearrange("p (t w k) -> p t w k", t=3, k=3)
                for ti in range(3):
                    ph1_act.append(
                        nc.scalar.activation(s3[:, ti, :, :],
                                             d3[:, :, 3 * ti:3 * ti + 3],
                                             Act.Square, scale=SSAFE))

                # r2 planar: [nc2 | na2 | nb2]  (windowed sums of squares)
                r2 = mid2.tile([P, 3 * W], dt.float32, tag="r2")
                r2m = cfg.get("r2_mode", "dred")
                if r2m == "gtt":
                    # two GpSimd adds on strided views (offloads the DVE)
                    for ti in range(3):
                        rt = r2[:, ti * W:(ti + 1) * W].rearrange(
                            "p (w o) -> p w o", o=1)
                        nc.gpsimd.tensor_tensor(
                            rt, s3[:, ti, :, 0:1], s3[:, ti, :, 1:2],
                            op=Alu.add)
                        nc.gpsimd.tensor_tensor(
                            rt, rt, s3[:, ti, :, 2:3], op=Alu.add)
                else:
                    for ti in range(3):
                        nc.vector.tensor_reduce(
                            r2[:, ti * W:(ti + 1) * W].rearrange(
                                "p (w o) -> p w o", o=1),
                            s3[:, ti, :, :], axis=mybir.AxisListType.X,
                            op=Alu.add)
                na2 = r2[:, W:2 * W]

                # p6[6j+m] = d[9j+m]*d[9j+m+3]  (m=0..2 -> dot2, 3..5 -> dot1)
                p6 = ppool.tile([P, 6 * W], dt.float32, tag="p6")
                p6v = p6[:, :].rearrange("p (w k) -> p w k", k=6)
                gfrac = cfg.get("p6_gps", 0.5)
                kp = int(round(W * gfrac))
                if kp > 0:
                    nc.gpsimd.tensor_tensor(p6v[:, 0:kp], d3[:, 0:kp, 0:6],
                                            d3[:, 0:kp, 3:9], op=Alu.mult)
                if kp < W:
                    nc.vector.tensor_tensor(p6v[:, kp:W], d3[:, kp:W, 0:6],
                                            d3[:, kp:W, 3:9], op=Alu.mult)

                # dots planar: [dot2 | dot1]
                pvv = p6[:, :].rearrange("p (w t k) -> p w t k", t=2, k=3)
                dots = mid2.tile([P, 2 * W], dt.float32, tag="dots")
                for ti in range(2):
                    nc.vector.tensor_reduce(
                        dots[:, ti * W:(ti + 1) * W].rearrange(
                            "p (w o) -> p w o", o=1),
                        pvv[:, :, ti, :], axis=mybir.AxisListType.X,
                        op=Alu.add)

                # q planar: [nc2*na2 | na2*nb2]  (dense overlapping slices)
                q = mid2.tile([P, 2 * W], dt.float32, tag="q")
                eng("q").tensor_tensor(q[:, :], r2[:, 0:2 * W],
                                       r2[:, W:3 * W], op=Alu.mult)

                # v = q - dots^2  (> 0 thanks to SSAFE)
                v = mid1.tile([P, 2 * W], dt.float32, tag="v")
                if fuse:
                    nc.vector._custom_dve(fuse["V_FUSE_ANT"], out=v[:, :],
                                          in0=q[:, :], in1=dots[:, :])
                else:
                    dsq = mid1.tile([P, 2 * W], dt.float32, tag="dsq")
                    eng("dsq").tensor_tensor(dsq[:, :], dots[:, :],
                                             dots[:, :], op=Alu.mult)
                    eng("v").tensor_tensor(v[:, :], q[:, :], dsq[:, :],
                                           op=Alu.subtract)

                # sqv = sqrt(v); rcp = 1/sqv (fast custom-DVE reciprocal)
                sqv = mid1.tile([P, 2 * W], dt.float32, tag="sqv")
                ph1_act.append(
                    nc.scalar.activation(sqv[:, :], v[:, :], Act.Sqrt,
                                         bias=VEPS))
                rcp = mid1.tile([P, 2 * W], dt.float32, tag="rcp")
                nc.vector.reciprocal_approx_fast(rcp[:, :], sqv[:, :])

                # cot = dots*rcp, clamped into the carry: [cc1 | cc2 | lp0]
                carry = xph.tile([P, 3 * W], dt.float32, tag="carry")
                if fuse:
                    nc.vector._custom_dve(
                        fuse["MUL_CLAMP_ANT"], out=carry[:, 0:W],
                        in0=dots[:, W:2 * W], in1=rcp[:, W:2 * W], s0=CLAMP)
                    nc.vector._custom_dve(
                        fuse["MUL_CLAMP_ANT"], out=carry[:, W:2 * W],
                        in0=dots[:, 0:W], in1=rcp[:, 0:W], s0=CLAMP)
                else:
                    cot = mid1.tile([P, 2 * W], dt.float32, tag="cot")
                    eng("cot").tensor_tensor(cot[:, :], dots[:, :], rcp[:, :],
                                             op=Alu.mult)
                    eng("cc").tensor_scalar(carry[:, 0:W], cot[:, W:2 * W],
                                            CLAMP, -CLAMP, op0=Alu.min,
                                            op1=Alu.max)
                    eng("cc").tensor_scalar(carry[:, W:2 * W], cot[:, 0:W],
                                            CLAMP, -CLAMP, op0=Alu.min,
                                            op1=Alu.max)

                # length term: na = sqrt(na2); d0sq = (sk0*na + b0)^2;
                # lp0 = min(d0sq, c0)  (max-with-0 is a free no-op 2nd op)
                na = mid1.tile([P, W], dt.float32, tag="na")
                ph1_act.append(
                    nc.scalar.activation(na[:, :], na2, Act.Sqrt))
                d0sq = mid1.tile([P, W], dt.float32, tag="d0sq")
                ph1_act.append(
                    nc.scalar.activation(d0sq[:, :], na[:, :], Act.Square,
                                         bias=con["b0"], scale=con["sk0"]))
                eng("lp").tensor_scalar(carry[:, 2 * W:3 * W], d0sq[:, :],
                                        con["c0"], 0.0, op0=Alu.min,
                                        op1=Alu.max)
                return carry

            def emit_ph2(t, carry):
                h = ph2.tile([P, 2 * W], dt.float32, tag="h")
                h_inst = nc.scalar.activation(h[:, :], carry[:, 0:2 * W],
                                              Act.Arctan)
                if fence:
                    for a in ph1_act:
                        bass_rust.add_dep_helper(
                            h_inst.ins, a.ins,
                            reason="act set fence: sqrt-set before arctan")
                # sq_i = (sk_i*h_i + b_i)^2  (h1 in [0:W], h2 in [W:2W])
                sq = ph2.tile([P, 2 * W], dt.float32, tag="sq")
                nc.scalar.activation(sq[:, 0:W], h[:, 0:W], Act.Square,
                                     bias=con["b1"], scale=con["sk1"])
                nc.scalar.activation(sq[:, W:2 * W], h[:, W:2 * W], Act.Square,
                                     bias=con["b2"], scale=con["sk2"])
                val = vpool.tile([P, W], dt.float32, tag="val")
                if fuse:
                    # val = min(sq2,c2) + (min(sq1,c1) + lp0)
                    t1 = ph2.tile([P, W], dt.float32, tag="t1")
                    nc.vector._custom_dve(
                        fuse["MIN_ADD_ANT"], out=t1[:, :], in0=sq[:, 0:W],
                        in1=carry[:, 2 * W:3 * W], s0=con["c1"])
                    nc.vector._custom_dve(
                        fuse["MIN_ADD_ANT"], out=val[:, :],
                        in0=sq[:, W:2 * W], in1=t1[:, :], s0=con["c2"])
                else:
                    lp = ph2.tile([P, 2 * W], dt.float32, tag="lp")
                    eng("lp").tensor_scalar(lp[:, 0:W], sq[:, 0:W], con["c1"],
                                            0.0, op0=Alu.min, op1=Alu.max)
                    eng("lp").tensor_scalar(lp[:, W:2 * W], sq[:, W:2 * W],
                                            con["c2"], 0.0, op0=Alu.min,
                                            op1=Alu.max)
                    s12 = ph2.tile([P, W], dt.float32, tag="s12")
                    eng("s12").tensor_tensor(s12[:, :], lp[:, 0:W],
                                             lp[:, W:2 * W], op=Alu.add)
                    eng("val").tensor_tensor(val[:, :], s12[:, :],
                                             carry[:, 2 * W:3 * W],
                                             op=Alu.add)
                dst = bass.AP(out, P * W * t, [[W, P], [1, W]])
                nc.sync.dma_start(dst, val[:, :])

            def _body():
                carries = []
                for t in range(NT):
                    carries.append(emit_ph1(t))
                for t in range(NT):
                    emit_ph2(t, carries[t])

            if reps == 1:
                _body()
            else:
                with tc.For_i(0, reps, 1):
                    _body()

    nc.compile()
    return nc


def _get_program(con, cfg=None):
    global _PROGRAM, _PROGRAM_KEY
    if cfg is None:
        cfg = CFG
    key = (tuple(sorted(con.items())), repr(sorted(cfg.items())))
    if _PROGRAM is None or _PROGRAM_KEY != key:
        _PROGRAM = _build_program(con, cfg=cfg)
        _PROGRAM_KEY = key
    return _PROGRAM


# ---------------------------------------------------------------- host side
def _is_canonical(ad, coords, mean, std):
    if ad.shape != (N_ATOMS, 5) or coords.shape != (N_ATOMS, 3):
        return False
    if mean.shape != (20, 3) or std.shape != (20, 3):
        return False
    if not (np.all(mean == mean[0:1]) and np.all(std == std[0:1])):
        return False
    if not np.all(std[0] > 0):
        return False
    a5 = ad.reshape(B, NCH, NRES, APR, 5)
    if not np.all(a5[..., 0] == np.arange(B, dtype=ad.dtype)[:, None, None, None]):
        return False
    if not np.all(a5[..., 1] == np.arange(NCH, dtype=ad.dtype)[:, None, None]):
        return False
    if not np.all(a5[..., 2] == np.arange(NRES, dtype=ad.dtype)[:, None]):
        return False
    if not np.all(a5[..., 4] == np.arange(APR, dtype=ad.dtype)):
        return False
    if not np.isfinite(coords).all() or np.abs(coords).max() >= 1e4:
        return False
    # all bond-geometry norms must clear the reference's EPS mask, so the
    # device kernel can skip mask arithmetic entirely
    r = coords.reshape(B, NCH, NRES, 9)
    w = r[:, :, 1:, 0:3] - r[:, :, :-1, 6:9]
    a = r[:, :, 1:, 3:6] - r[:, :, 1:, 0:3]
    bb = r[:, :, :-1, 6:9] - r[:, :, :-1, 3:6]
    mn = min(
        (w * w).sum(-1).min(),
        (a * a).sum(-1).min(),
        (bb * bb).sum(-1).min(),
    )
    return bool(mn > 1.1e-16)


def _run_fast(coords, mean, std, weight, n_alt):
    from concourse import bass_utils

    con = _consts(mean[0], std[0], weight[0])
    nc = _get_program(con)
    need_xng = bool(CFG.get("dma_d_tiles"))
    cflat = np.ascontiguousarray(coords.reshape(-1), dtype=np.float32)
    in_maps = []
    for c in range(NCORES):
        shard = np.empty(((RES_PER_CORE + 1) * 9,), dtype=np.float32)
        shard[:-9] = cflat[c * ATOMS_PER_CORE * 3:(c + 1) * ATOMS_PER_CORE * 3]
        shard[-9:] = _PAD_RESIDUE
        m = {"xin": shard}
        if need_xng:
            m["xng"] = -shard
        in_maps.append(m)

    res = bass_utils.run_bass_kernel_spmd(nc, in_maps,
                                          core_ids=list(range(NCORES)))
    parts = [np.asarray(res.results[c]["out"], dtype=np.float32)
             for c in range(NCORES)]
    e = np.concatenate(parts).reshape(B, NCH, NRES)
    e[:, :, NRES - 1] = 0.0          # no bond out of the last residue
    full = np.zeros((B, NCH, NRES, n_alt), dtype=np.float32)
    full[..., 0] = e
    return full


# ------------------------------------------------------------ numpy fallback
def _fallback(ad, coords, alternatives, weight, mean, std):
    """Faithful numpy port of the jax reference (incl. OOB drop/clamp)."""
    n_alt = alternatives.shape[-1]
    batch, chain, resnum = ad[:, 0], ad[:, 1], ad[:, 2]
    resname, at_name = ad[:, 3], ad[:, 4]
    n = ad.shape[0]

    table = np.full((B, NCH, NRES, APR), -1, dtype=np.int32)
    ok = ((batch >= 0) & (batch < B) & (chain >= 0) & (chain < NCH)
          & (resnum >= 0) & (resnum < NRES) & (at_name >= 0) & (at_name < APR))
    idx = np.arange(n, dtype=np.int32)
    table[batch[ok], chain[ok], resnum[ok], at_name[ok]] = idx[ok]

    c_idx = table[:, :, :-1, 2].reshape(-1)
    n_idx = table[:, :, 1:, 0].reshape(-1)
    cac_idx = table[:, :, :-1, 1].reshape(-1)
    can_idx = table[:, :, 1:, 1].reshape(-1)
    valid_idx = (c_idx >= 0) & (n_idx >= 0) & (cac_idx >= 0) & (can_idx >= 0)
    safe = lambda i: np.where(i >= 0, i, 0)

    co = coords.astype(np.float32)
    c_xyz = co[safe(c_idx)]
    n_xyz = co[safe(n_idx)]
    cac_xyz = co[safe(cac_idx)]
    can_xyz = co[safe(can_idx)]

    v_cn = n_xyz - c_xyz
    v_nca = can_xyz - n_xyz
    v_cac = c_xyz - cac_xyz

    def ang_deg(a, b):
        na = np.sqrt((a * a).sum(-1))
        nb = np.sqrt((b * b).sum(-1))
        mask = (na > EPS) & (nb > EPS)
        cos = np.clip((a * b).sum(-1) / (na * nb + EPS), -1.0, 1.0)
        return np.degrees(np.arccos(cos)).astype(np.float32), mask

    ang1, m1 = ang_deg(v_cn, v_nca)
    ang2, m2 = ang_deg(v_cac, -v_cn)
    bond_len = np.sqrt((v_cn * v_cn).sum(-1))
    valid = valid_idx & m1 & m2

    geom = np.stack([bond_len, ang1, ang2], axis=-1)
    seq = np.clip(resname[safe(c_idx)], 0, 19)
    var = (std.astype(np.float32)[seq]) ** 2
    denom = np.sqrt(2.0 * np.pi * var).astype(np.float32)
    num = np.exp(-((geom - mean.astype(np.float32)[seq]) ** 2) / (2.0 * var))
    log_prob = -(np.log(np.clip(num / denom, EPS, None)) + np.log(denom))
    scores = log_prob.sum(-1)

    f = np.float32(1.0 - np.tanh(-np.float32(weight[0])))
    val = np.where(valid, scores * f, 0.0).astype(np.float32)

    b_c = batch[safe(c_idx)]
    ch_c = chain[safe(c_idx)]
    r_c = resnum[safe(c_idx)]
    resi = np.zeros((B, NCH, NRES, n_alt), dtype=np.float32)
    ok2 = ((b_c >= 0) & (b_c < B) & (ch_c >= 0) & (ch_c < NCH)
           & (r_c >= 0) & (r_c < NRES))
    resi[b_c[ok2], ch_c[ok2], r_c[ok2], 0] = val[ok2]
    return resi


# ----------------------------------------------------------------- entry
def kernel(atom_description, coords, alternatives, weight, mean, std):
    ad = np.asarray(atom_description)
    co = np.asarray(coords, dtype=np.float32)
    al = np.asarray(alternatives)
    wt = np.asarray(weight, dtype=np.float32)
    mn = np.asarray(mean, dtype=np.float32)
    sd = np.asarray(std, dtype=np.float32)

    if _is_canonical(ad, co, mn, sd):
        return _run_fast(co, mn, sd, wt, al.shape[-1])
    return _fallback(ad, co, al, wt, mn, sd)
